# revision 4
# baseline (speedup 1.0000x reference)
"""Trainium2 Bass kernel for cubic (Keys) interpolation of vertices in a 3D volume.

v3: bf16 shingle + slab-bucketed dma_gather + 2x-mode DVE pipeline.

Sharding: vertices are sorted by shingle row (host side) and split into 8
equal rank-ranges, one per NeuronCore. The volume is stored per core as a
bf16 shingle
    S[x, y, z, c, dx, dy] = vol[c, x+dx, y+dy, z]     (rows of 128 bf16)
with only the row-ranges that core's vertices touch, repacked into NSLAB
fixed-stride slabs so every gather index fits int16 (dma_gather's index
dtype). Rows (x,y,z)..(x,y,z+3) -- one 1KB run -- hold a vertex's whole
4x4x4x8 neighborhood in payload order [k(z):4, c:8, i(dx):4, j(dy):4];
channel sits mid-payload so the weight broadcast keeps innermost step=1 and
all large DVE ops run in 2x_1P bf16 mode.

Per core: NSLAB dma_gather calls (ONE SWDGE instruction each: ~1us + 1024
descriptors at 0.34ns) fetch 1024 vertex neighborhoods per call, landing
index j at (partition j%128, column j//128). Groups are padded to exactly
1024 with fake vertices so the call structure is SPMD-uniform; fake outputs
are dropped at reassembly. Weights (Keys cubic, via the raw-factor + 1/8
trick) are built per batch: outer products on GPSIMD (fp32), cast to bf16 on
ScalarE, multiply + 3 tree-reductions on DVE (2x mode), strided compact to
f32 on ScalarE, DMA out.
"""

import numpy as np
import ml_dtypes

import concourse.bass as bass
import concourse.tile as tile
from concourse import bacc, mybir
from concourse.bass_utils import run_bass_kernel_spmd

X, Y, Z, C = 112, 224, 160, 8
P = 128
NCORES = 8
V = 150000
VCORE = V // NCORES          # 18750
GROUP = 1024                 # indices per dma_gather call (8 columns)
GCOLS = GROUP // P           # 8
SPAN_CAP = 32700             # max row span within one slab (int16 margin)
BSTRIDE = 32772              # rows per slab in the repacked shingle
ES = 512                     # elems gathered per index (4 rows x 128)
STEP = 128                   # elems per row
TILE_SLABS = 4               # slabs processed per DVE batch
MAGIC = 12582912.0           # 1.5 * 2**23 fp32 round-to-int magic

BF16 = mybir.dt.bfloat16
F32 = mybir.dt.float32
I16 = mybir.dt.int16
ALU = mybir.AluOpType
ACT = mybir.ActivationFunctionType

_CACHE = {}


# --------------------------------------------------------------------------
# device program
# --------------------------------------------------------------------------

def _build_program(nslab):
    m = nslab * GCOLS  # total slot-columns per partition
    srows = nslab * BSTRIDE
    nc = bacc.Bacc("TRN2", target_bir_lowering=False, debug=False,
                   num_devices=NCORES)
    s_in = nc.dram_tensor("shingle", [srows, P], BF16, kind="ExternalInput").ap()
    vert_in = nc.dram_tensor("vert", [P, m * 3], F32, kind="ExternalInput").ap()
    idx_in = nc.dram_tensor("idx", [P, nslab * (GROUP // 16)], I16,
                            kind="ExternalInput").ap()
    out_ext = nc.dram_tensor("out", [P, m * C], F32, kind="ExternalOutput").ap()

    with tile.TileContext(nc) as tc:
        _emit(tc, nslab, out_ext, vert_in, idx_in, s_in)
    nc.compile()
    return nc


def _emit(tc, nslab, out_ext, vert_in, idx_in, s_in):
    nc = tc.nc
    vec = nc.vector
    m = nslab * GCOLS

    with (
        tc.tile_pool(name="keep", bufs=1) as keep,
        tc.tile_pool(name="pro", bufs=1) as pro,
        tc.tile_pool(name="gpool", bufs=3) as gpool,
        tc.tile_pool(name="wpool", bufs=2) as wpool,
        tc.tile_pool(name="opool", bufs=2) as opool,
    ):
        wr = keep.tile([P, 4 * m * 3], F32)   # raw weights [i, s, d]
        idx = keep.tile([P, nslab * (GROUP // 16)], I16)
        vt = pro.tile([P, m * 3], F32)
        fl = pro.tile([P, m * 3], F32)
        u = pro.tile([P, m * 3], F32)
        u2 = pro.tile([P, m * 3], F32)
        u3 = pro.tile([P, m * 3], F32)
        tmp = pro.tile([P, m * 3], F32)

        nc.sync.dma_start(out=idx[:], in_=idx_in)
        nc.sync.dma_start(out=vt[:], in_=vert_in)

        # clip per dim (max_b differs per dim)
        vt3 = vt[:].rearrange("p (s d) -> p s d", d=3)
        for d, dim in enumerate((X, Y, Z)):
            sl = vt3[:, :, d]
            vec.tensor_scalar(out=sl, in0=sl,
                              scalar1=float(np.float32(1.0 + 1e-5)),
                              scalar2=float(np.float32(dim - 2 - 1e-5)),
                              op0=ALU.max, op1=ALU.min)

        # fl = round(v - 0.5) via magic number (== floor except exact-int v,
        # where u becomes 1.0 and the window shifts by one -- same result;
        # the host used the identical computation for the gather indices)
        vec.tensor_scalar(out=fl[:], in0=vt[:], scalar1=0.5, scalar2=MAGIC,
                          op0=ALU.subtract, op1=ALU.add)
        vec.tensor_scalar(out=fl[:], in0=fl[:], scalar1=MAGIC, scalar2=None,
                          op0=ALU.subtract)

        vec.tensor_tensor(out=u[:], in0=vt[:], in1=fl[:], op=ALU.subtract)
        vec.tensor_tensor(out=u2[:], in0=u[:], in1=u[:], op=ALU.mult)
        vec.tensor_tensor(out=u3[:], in0=u2[:], in1=u[:], op=ALU.mult)

        # raw weights (2x the Keys weights; the 3 raw factors carry 8x,
        # compensated by folding 0.125 into the z weights below)
        wr4 = wr[:].rearrange("p (i e) -> p i e", i=4)
        w0, w1, w2, w3 = (wr4[:, i] for i in range(4))
        vec.tensor_tensor(out=tmp[:], in0=u3[:], in1=u[:], op=ALU.add)
        vec.scalar_tensor_tensor(out=w0, in0=u2[:], scalar=2.0, in1=tmp[:],
                                 op0=ALU.mult, op1=ALU.subtract)
        vec.tensor_scalar(out=tmp[:], in0=u2[:], scalar1=5.0, scalar2=2.0,
                          op0=ALU.mult, op1=ALU.subtract)
        vec.scalar_tensor_tensor(out=w1, in0=u3[:], scalar=3.0, in1=tmp[:],
                                 op0=ALU.mult, op1=ALU.subtract)
        vec.scalar_tensor_tensor(out=tmp[:], in0=u2[:], scalar=4.0, in1=u[:],
                                 op0=ALU.mult, op1=ALU.add)
        vec.scalar_tensor_tensor(out=w2, in0=u3[:], scalar=-3.0, in1=tmp[:],
                                 op0=ALU.mult, op1=ALU.add)
        vec.tensor_tensor(out=w3, in0=u3[:], in1=u2[:], op=ALU.subtract)
        wr_isd = wr[:].rearrange("p (i s d) -> p i s d", i=4, s=m, d=3)
        wz_all = wr_isd[:, :, :, 2]
        vec.tensor_scalar(out=wz_all, in0=wz_all, scalar1=0.125, scalar2=None,
                          op0=ALU.mult)

        batches = []
        b0 = 0
        while b0 < nslab:
            batches.append((b0, min(b0 + TILE_SLABS, nslab)))
            b0 += TILE_SLABS

        for (b0, b1) in batches:
            nb = b1 - b0
            ns = nb * GCOLS          # slots this batch
            s0 = b0 * GCOLS
            G = gpool.tile([P, TILE_SLABS * GCOLS * ES], BF16, tag="G")
            A = wpool.tile([P, TILE_SLABS * GCOLS * 16], F32, tag="A")
            Wf = wpool.tile([P, TILE_SLABS * GCOLS * 64], F32, tag="Wf")
            Wb = wpool.tile([P, TILE_SLABS * GCOLS * 64], BF16, tag="Wb")
            ot = opool.tile([P, TILE_SLABS * GCOLS * C], F32, tag="ot")

            # one dma_gather per slab: 1024 indices x 1KB runs
            for b in range(b0, b1):
                src_win = bass.AP(s_in.tensor, b * BSTRIDE * STEP,
                                  [[STEP, SPAN_CAP + 8], [1, ES]])
                gv = G[:, (b - b0) * GCOLS * ES:(b - b0 + 1) * GCOLS * ES] \
                    .rearrange("p (t e) -> p t e", e=ES)
                nc.gpsimd.dma_gather(
                    out_ap=gv, in_ap=src_win,
                    idxs_ap=idx[:, b * (GROUP // 16):(b + 1) * (GROUP // 16)],
                    num_idxs=GROUP, num_idxs_reg=GROUP,
                    elem_size=ES, elem_step=STEP)

            wz = wr_isd[:, :, s0:s0 + ns, 2].transpose([0, 2, 1])
            wx = wr_isd[:, :, s0:s0 + ns, 0].transpose([0, 2, 1])
            wy = wr_isd[:, :, s0:s0 + ns, 1].transpose([0, 2, 1])

            # weight outer products on GPSIMD (fp32) to keep DVE free
            Av = A[:, :ns * 16].rearrange("p (s k i) -> p s k i", s=ns, k=4, i=4)
            nc.gpsimd.tensor_tensor(
                out=Av,
                in0=wz.unsqueeze(3).to_broadcast([P, ns, 4, 4]),
                in1=wx.unsqueeze(2).to_broadcast([P, ns, 4, 4]),
                op=ALU.mult)
            Wv = Wf[:, :ns * 64].rearrange("p (s e j) -> p s e j", s=ns, e=16, j=4)
            nc.gpsimd.tensor_tensor(
                out=Wv,
                in0=A[:, :ns * 16].rearrange("p (s e) -> p s e", s=ns)
                    .unsqueeze(3).to_broadcast([P, ns, 16, 4]),
                in1=wy.unsqueeze(2).to_broadcast([P, ns, 16, 4]),
                op=ALU.mult)
            # cast to bf16 on ScalarE
            nc.scalar.activation(out=Wb[:, :ns * 64], in_=Wf[:, :ns * 64],
                                 func=ACT.Copy)

            # G *= W  (payload [k, c, i, j]; c broadcast mid-dim -> 2x)
            Gv = G[:, :ns * ES].rearrange("p (s k c e) -> p s k c e",
                                          s=ns, k=4, c=8, e=16)
            vec.tensor_tensor(
                out=Gv, in0=Gv,
                in1=Wb[:, :ns * 64].rearrange("p (s k e) -> p s k e", s=ns, k=4)
                    .unsqueeze(3).to_broadcast([P, ns, 4, 8, 16]),
                op=ALU.mult)

            # k-tree (stride 128), 2 ops
            Gk = G[:, :ns * ES].rearrange("p (s k r) -> p s k r",
                                          s=ns, k=4, r=128)
            vec.tensor_tensor(out=Gk[:, :, 0:2], in0=Gk[:, :, 0:2],
                              in1=Gk[:, :, 2:4], op=ALU.add)
            vec.tensor_tensor(out=Gk[:, :, 0:1], in0=Gk[:, :, 0:1],
                              in1=Gk[:, :, 1:2], op=ALU.add)
            # i-tree, 2 ops
            Gi = G[:, :ns * ES].rearrange("p (s k c i j) -> p s k c i j",
                                          s=ns, k=4, c=8, i=4, j=4)[:, :, 0]
            vec.tensor_tensor(out=Gi[:, :, :, 0:2], in0=Gi[:, :, :, 0:2],
                              in1=Gi[:, :, :, 2:4], op=ALU.add)
            vec.tensor_tensor(out=Gi[:, :, :, 0:1], in0=Gi[:, :, :, 0:1],
                              in1=Gi[:, :, :, 1:2], op=ALU.add)
            # j-tree, 2 ops
            Gj = Gi[:, :, :, 0]
            vec.tensor_tensor(out=Gj[:, :, :, 0:2], in0=Gj[:, :, :, 0:2],
                              in1=Gj[:, :, :, 2:4], op=ALU.add)
            vec.tensor_tensor(out=Gj[:, :, :, 0:1], in0=Gj[:, :, :, 0:1],
                              in1=Gj[:, :, :, 1:2], op=ALU.add)

            # compact strided [s, c] (stride 16) -> f32 on ScalarE
            nc.scalar.activation(
                out=ot[:, :ns * C].rearrange("p (s c) -> p s c", c=C),
                in_=Gj[:, :, :, 0], func=ACT.Copy)
            nc.sync.dma_start(out=out_ext[:, s0 * C:(s0 + ns) * C],
                              in_=ot[:, :ns * C])


def _get_program(nslab):
    key = ("nc", nslab)
    if key not in _CACHE:
        _CACHE[key] = _build_program(nslab)
    return _CACHE[key]


# --------------------------------------------------------------------------
# host-side preparation
# --------------------------------------------------------------------------

def _f32_to_bf16_bits(a):
    b = a.view(np.uint32)
    rounded = b + 0x7FFF + ((b >> 16) & 1)
    return (rounded >> 16).astype(np.uint16)


def _build_shingle_u16(vol):
    """S[x, y, z, c, dx, dy] = vol[c, x+dx, y+dy, z], flat [NROW, 128] u16."""
    v = np.ascontiguousarray(np.asarray(vol[0], dtype=np.float32))  # (C,X,Y,Z)
    vb = _f32_to_bf16_bits(v)
    vt = np.ascontiguousarray(vb.transpose(1, 2, 3, 0))             # (X,Y,Z,C)
    S = np.zeros((X, Y, Z, C, 4, 4), np.uint16)
    for dx in range(4):
        for dy in range(4):
            S[:X - dx, :Y - dy, :, :, dx, dy] = vt[dx:, dy:, :, :]
    return S.reshape(X * Y * Z, 128)


def _host_rows(vert):
    """Exact replica of the device clip/floor -> shingle row per vertex."""
    v = np.asarray(vert[0], dtype=np.float32)
    vc = np.empty_like(v)
    for d, dim in enumerate((X, Y, Z)):
        vc[:, d] = np.clip(v[:, d], np.float32(1.0 + 1e-5),
                           np.float32(dim - 2 - 1e-5))
    mg = np.float32(MAGIC)
    fl = ((vc - np.float32(0.5)) + mg) - mg
    fli = fl.astype(np.int64)
    return ((fli[:, 0] - 1) * Y + (fli[:, 1] - 1)) * Z + (fli[:, 2] - 1)


def _prepare(vert, vol):
    rows = _host_rows(vert)                      # (V,)
    order = np.argsort(rows, kind="stable")
    Sfull = _build_shingle_u16(vol)

    # per-core grouping
    cores = []
    nslab_needed = 0
    for c in range(NCORES):
        ids = order[c * VCORE:(c + 1) * VCORE]
        r = rows[ids]
        groups = []
        i = 0
        n = len(ids)
        while i < n:
            jmax = min(i + GROUP, n)
            j = int(np.searchsorted(r, r[i] + SPAN_CAP, side="right"))
            j = min(j, jmax)
            groups.append((i, j))
            i = j
        cores.append((ids, r, groups))
        nslab_needed = max(nslab_needed, len(groups))
    nslab = nslab_needed
    m = nslab * GCOLS
    srows = nslab * BSTRIDE

    vert_f = np.asarray(vert[0], dtype=np.float32)
    in_maps = []
    perms = []   # per core: (orig_ids, flat positions in out)
    for c in range(NCORES):
        ids, r, groups = cores[c]
        sh = np.zeros((srows, P), np.uint16)
        vpad = np.full((m * P, 3), 1.5, np.float32)
        idx16 = np.zeros((nslab, 16, GROUP // 16), np.int16)
        flat_pos = np.empty(len(ids), np.int64)
        for b in range(nslab):
            if b < len(groups):
                i, j = groups[b]
                r0 = int(r[i])
                r1 = int(r[j - 1])
                nrow = min(r1 - r0 + 4, srows - b * BSTRIDE)
                sh[b * BSTRIDE:b * BSTRIDE + nrow] = Sfull[r0:r0 + nrow]
                rel = (r[i:j] - r0).astype(np.int16)
                cnt = j - i
            else:
                rel = np.zeros(0, np.int16)
                cnt = 0
            jj = np.arange(cnt)
            # index position within call: wrapped [16, GROUP//16]
            full_rel = np.zeros(GROUP, np.int16)
            full_rel[:cnt] = rel
            idx16[b] = full_rel.reshape(GROUP // 16, 16).T
            if cnt:
                gi = groups[b][0]
                # vertex ids[gi + jj] lands at partition jj%128, col b*8+jj//128
                part = jj % P
                col = b * GCOLS + jj // P
                flat_pos[gi:gi + cnt] = part * m + col
                vpad[part * m + col] = vert_f[ids[gi:gi + cnt]]
        in_maps.append({
            "shingle": sh.view(ml_dtypes.bfloat16),
            "vert": np.ascontiguousarray(vpad.reshape(P, m * 3)),
            "idx": np.ascontiguousarray(
                np.tile(idx16.transpose(1, 0, 2).reshape(16, -1), (8, 1))),
        })
        perms.append((ids, flat_pos))
    return nslab, in_maps, perms


def run_cores(vert, vol, trace=False, n_cores=NCORES, **kwargs):
    nslab, in_maps, perms = _prepare(vert, vol)
    nc = _get_program(nslab)
    res = run_bass_kernel_spmd(nc, in_maps, list(range(n_cores)),
                               trace=trace, **kwargs)
    m = nslab * GCOLS
    full = np.zeros((1, V, C), np.float32)
    for c in range(n_cores):
        out = np.asarray(res.results[c]["out"]).reshape(P * m, C)
        ids, flat_pos = perms[c]
        full[0, ids] = out[flat_pos]
    return full, res


def kernel(vert, vol):
    full, _ = run_cores(vert, vol, trace=False)
    return full


# revision 7
# speedup vs baseline: 2.1523x; 2.1523x over previous
"""Trainium2 Bass kernel for cubic (Keys) interpolation of vertices in a 3D volume.

v3: bf16 shingle + slab-bucketed dma_gather + 2x-mode DVE pipeline.

Sharding: vertices are sorted by shingle row (host side) and split into 8
equal rank-ranges, one per NeuronCore. The volume is stored per core as a
bf16 shingle
    S[x, y, z, c, dx, dy] = vol[c, x+dx, y+dy, z]     (rows of 128 bf16)
with only the row-ranges that core's vertices touch, repacked into NSLAB
fixed-stride slabs so every gather index fits int16 (dma_gather's index
dtype). Rows (x,y,z)..(x,y,z+3) -- one 1KB run -- hold a vertex's whole
4x4x4x8 neighborhood in payload order [k(z):4, c:8, i(dx):4, j(dy):4];
channel sits mid-payload so the weight broadcast keeps innermost step=1 and
all large DVE ops run in 2x_1P bf16 mode.

Per core: NSLAB dma_gather calls (ONE SWDGE instruction each: ~1us + 1024
descriptors at 0.34ns) fetch 1024 vertex neighborhoods per call, landing
index j at (partition j%128, column j//128). Groups are padded to exactly
1024 with fake vertices so the call structure is SPMD-uniform; fake outputs
are dropped at reassembly. Weights (Keys cubic, via the raw-factor + 1/8
trick) are built per batch: outer products on GPSIMD (fp32), cast to bf16 on
ScalarE, multiply + 3 tree-reductions on DVE (2x mode), strided compact to
f32 on ScalarE, DMA out.
"""

import numpy as np
import ml_dtypes

import concourse.bass as bass
import concourse.tile as tile
from concourse import bacc, mybir
from concourse.bass_utils import run_bass_kernel_spmd

X, Y, Z, C = 112, 224, 160, 8
P = 128
NCORES = 8
V = 150000
VCORE = V // NCORES          # 18750
GROUP = 1024                 # indices per dma_gather call (8 columns)
GCOLS = GROUP // P           # 8
SPAN_CAP = 32700             # max row span within one slab (int16 margin)
BSTRIDE = 32772              # rows per slab in the repacked shingle
ES = 512                     # elems gathered per index (4 rows x 128)
STEP = 128                   # elems per row
TILE_SLABS = 4               # slabs processed per DVE batch
MAGIC = 12582912.0           # 1.5 * 2**23 fp32 round-to-int magic

BF16 = mybir.dt.bfloat16
F32 = mybir.dt.float32
I16 = mybir.dt.int16
ALU = mybir.AluOpType
ACT = mybir.ActivationFunctionType

_CACHE = {}


# --------------------------------------------------------------------------
# device program
# --------------------------------------------------------------------------

def _build_program(nslab):
    m = nslab * GCOLS  # total slot-columns per partition
    srows = nslab * BSTRIDE
    nc = bacc.Bacc("TRN2", target_bir_lowering=False, debug=False,
                   num_devices=NCORES, num_swdge_queues=4)
    s_in = nc.dram_tensor("shingle", [srows, P], BF16, kind="ExternalInput").ap()
    vert_in = nc.dram_tensor("vert", [P, m * 3], F32, kind="ExternalInput").ap()
    idx_in = nc.dram_tensor("idx", [P, nslab * (GROUP // 16)], I16,
                            kind="ExternalInput").ap()
    out_ext = nc.dram_tensor("out", [P, m * C], F32, kind="ExternalOutput").ap()

    with tile.TileContext(nc) as tc:
        _emit(tc, nslab, out_ext, vert_in, idx_in, s_in)
    nc.compile()
    return nc


def _emit(tc, nslab, out_ext, vert_in, idx_in, s_in):
    nc = tc.nc
    vec = nc.vector
    m = nslab * GCOLS

    with (
        tc.tile_pool(name="keep", bufs=1) as keep,
        tc.tile_pool(name="pro", bufs=1) as pro,
        tc.tile_pool(name="gpool", bufs=3) as gpool,
        tc.tile_pool(name="wpool", bufs=2) as wpool,
        tc.tile_pool(name="opool", bufs=2) as opool,
    ):
        wr = keep.tile([P, 4 * m * 3], F32)   # raw weights [i, s, d]
        idx = keep.tile([P, nslab * (GROUP // 16)], I16)
        vt = pro.tile([P, m * 3], F32)
        fl = pro.tile([P, m * 3], F32)
        u = pro.tile([P, m * 3], F32)
        u2 = pro.tile([P, m * 3], F32)
        u3 = pro.tile([P, m * 3], F32)
        tmp = pro.tile([P, m * 3], F32)

        nc.sync.dma_start(out=idx[:], in_=idx_in)
        nc.sync.dma_start(out=vt[:], in_=vert_in)

        # clip per dim (max_b differs per dim)
        vt3 = vt[:].rearrange("p (s d) -> p s d", d=3)
        for d, dim in enumerate((X, Y, Z)):
            sl = vt3[:, :, d]
            vec.tensor_scalar(out=sl, in0=sl,
                              scalar1=float(np.float32(1.0 + 1e-5)),
                              scalar2=float(np.float32(dim - 2 - 1e-5)),
                              op0=ALU.max, op1=ALU.min)

        # fl = round(v - 0.5) via magic number (== floor except exact-int v,
        # where u becomes 1.0 and the window shifts by one -- same result;
        # the host used the identical computation for the gather indices)
        vec.tensor_scalar(out=fl[:], in0=vt[:], scalar1=0.5, scalar2=MAGIC,
                          op0=ALU.subtract, op1=ALU.add)
        vec.tensor_scalar(out=fl[:], in0=fl[:], scalar1=MAGIC, scalar2=None,
                          op0=ALU.subtract)

        vec.tensor_tensor(out=u[:], in0=vt[:], in1=fl[:], op=ALU.subtract)
        vec.tensor_tensor(out=u2[:], in0=u[:], in1=u[:], op=ALU.mult)
        vec.tensor_tensor(out=u3[:], in0=u2[:], in1=u[:], op=ALU.mult)

        # raw weights (2x the Keys weights; the 3 raw factors carry 8x,
        # compensated by folding 0.125 into the z weights below)
        wr4 = wr[:].rearrange("p (i e) -> p i e", i=4)
        w0, w1, w2, w3 = (wr4[:, i] for i in range(4))
        vec.tensor_tensor(out=tmp[:], in0=u3[:], in1=u[:], op=ALU.add)
        vec.scalar_tensor_tensor(out=w0, in0=u2[:], scalar=2.0, in1=tmp[:],
                                 op0=ALU.mult, op1=ALU.subtract)
        vec.tensor_scalar(out=tmp[:], in0=u2[:], scalar1=5.0, scalar2=2.0,
                          op0=ALU.mult, op1=ALU.subtract)
        vec.scalar_tensor_tensor(out=w1, in0=u3[:], scalar=3.0, in1=tmp[:],
                                 op0=ALU.mult, op1=ALU.subtract)
        vec.scalar_tensor_tensor(out=tmp[:], in0=u2[:], scalar=4.0, in1=u[:],
                                 op0=ALU.mult, op1=ALU.add)
        vec.scalar_tensor_tensor(out=w2, in0=u3[:], scalar=-3.0, in1=tmp[:],
                                 op0=ALU.mult, op1=ALU.add)
        vec.tensor_tensor(out=w3, in0=u3[:], in1=u2[:], op=ALU.subtract)
        wr_isd = wr[:].rearrange("p (i s d) -> p i s d", i=4, s=m, d=3)
        wz_all = wr_isd[:, :, :, 2]
        vec.tensor_scalar(out=wz_all, in0=wz_all, scalar1=0.125, scalar2=None,
                          op0=ALU.mult)

        batches = []
        b0 = 0
        while b0 < nslab:
            batches.append((b0, min(b0 + TILE_SLABS, nslab)))
            b0 += TILE_SLABS

        for (b0, b1) in batches:
            nb = b1 - b0
            ns = nb * GCOLS          # slots this batch
            s0 = b0 * GCOLS
            G = gpool.tile([P, TILE_SLABS * GCOLS * ES], BF16, tag="G")
            A = wpool.tile([P, TILE_SLABS * GCOLS * 16], F32, tag="A")
            Wf = wpool.tile([P, TILE_SLABS * GCOLS * 64], F32, tag="Wf")
            Wb = wpool.tile([P, TILE_SLABS * GCOLS * 64], BF16, tag="Wb")
            ot = opool.tile([P, TILE_SLABS * GCOLS * C], F32, tag="ot")

            # one dma_gather per slab: 1024 indices x 1KB runs
            for b in range(b0, b1):
                src_win = bass.AP(s_in.tensor, b * BSTRIDE * STEP,
                                  [[STEP, SPAN_CAP + 8], [1, ES]])
                gv = G[:, (b - b0) * GCOLS * ES:(b - b0 + 1) * GCOLS * ES] \
                    .rearrange("p (t e) -> p t e", e=ES)
                nc.gpsimd.dma_gather(
                    out_ap=gv, in_ap=src_win,
                    idxs_ap=idx[:, b * (GROUP // 16):(b + 1) * (GROUP // 16)],
                    num_idxs=GROUP, num_idxs_reg=GROUP,
                    elem_size=ES, elem_step=STEP, queue_num=b % 4)

            wz = wr_isd[:, :, s0:s0 + ns, 2].transpose([0, 2, 1])
            wx = wr_isd[:, :, s0:s0 + ns, 0].transpose([0, 2, 1])
            wy = wr_isd[:, :, s0:s0 + ns, 1].transpose([0, 2, 1])

            # weight outer products (fp32, DVE 1x -- GPSIMD runs these ~10x
            # slower on broadcast access patterns)
            Av = A[:, :ns * 16].rearrange("p (s k i) -> p s k i", s=ns, k=4, i=4)
            vec.tensor_tensor(
                out=Av,
                in0=wz.unsqueeze(3).to_broadcast([P, ns, 4, 4]),
                in1=wx.unsqueeze(2).to_broadcast([P, ns, 4, 4]),
                op=ALU.mult)
            Wv = Wf[:, :ns * 64].rearrange("p (s e j) -> p s e j", s=ns, e=16, j=4)
            vec.tensor_tensor(
                out=Wv,
                in0=A[:, :ns * 16].rearrange("p (s e) -> p s e", s=ns)
                    .unsqueeze(3).to_broadcast([P, ns, 16, 4]),
                in1=wy.unsqueeze(2).to_broadcast([P, ns, 16, 4]),
                op=ALU.mult)
            # cast to bf16 on ScalarE
            nc.scalar.activation(out=Wb[:, :ns * 64], in_=Wf[:, :ns * 64],
                                 func=ACT.Copy)

            # G *= W  (payload [k, c, i, j]; c broadcast mid-dim -> 2x)
            Gv = G[:, :ns * ES].rearrange("p (s k c e) -> p s k c e",
                                          s=ns, k=4, c=8, e=16)
            vec.tensor_tensor(
                out=Gv, in0=Gv,
                in1=Wb[:, :ns * 64].rearrange("p (s k e) -> p s k e", s=ns, k=4)
                    .unsqueeze(3).to_broadcast([P, ns, 4, 8, 16]),
                op=ALU.mult)

            # k-tree (stride 128), 2 ops
            Gk = G[:, :ns * ES].rearrange("p (s k r) -> p s k r",
                                          s=ns, k=4, r=128)
            vec.tensor_tensor(out=Gk[:, :, 0:2], in0=Gk[:, :, 0:2],
                              in1=Gk[:, :, 2:4], op=ALU.add)
            vec.tensor_tensor(out=Gk[:, :, 0:1], in0=Gk[:, :, 0:1],
                              in1=Gk[:, :, 1:2], op=ALU.add)
            # i-tree, 2 ops
            Gi = G[:, :ns * ES].rearrange("p (s k c i j) -> p s k c i j",
                                          s=ns, k=4, c=8, i=4, j=4)[:, :, 0]
            vec.tensor_tensor(out=Gi[:, :, :, 0:2], in0=Gi[:, :, :, 0:2],
                              in1=Gi[:, :, :, 2:4], op=ALU.add)
            vec.tensor_tensor(out=Gi[:, :, :, 0:1], in0=Gi[:, :, :, 0:1],
                              in1=Gi[:, :, :, 1:2], op=ALU.add)
            # j-tree, 2 ops
            Gj = Gi[:, :, :, 0]
            vec.tensor_tensor(out=Gj[:, :, :, 0:2], in0=Gj[:, :, :, 0:2],
                              in1=Gj[:, :, :, 2:4], op=ALU.add)
            vec.tensor_tensor(out=Gj[:, :, :, 0:1], in0=Gj[:, :, :, 0:1],
                              in1=Gj[:, :, :, 1:2], op=ALU.add)

            # compact strided [s, c] (stride 16) -> f32 on ScalarE
            nc.scalar.activation(
                out=ot[:, :ns * C].rearrange("p (s c) -> p s c", c=C),
                in_=Gj[:, :, :, 0], func=ACT.Copy)
            nc.sync.dma_start(out=out_ext[:, s0 * C:(s0 + ns) * C],
                              in_=ot[:, :ns * C])


def _get_program(nslab):
    key = ("nc", nslab)
    if key not in _CACHE:
        _CACHE[key] = _build_program(nslab)
    return _CACHE[key]


# --------------------------------------------------------------------------
# host-side preparation
# --------------------------------------------------------------------------

def _f32_to_bf16_bits(a):
    b = a.view(np.uint32)
    rounded = b + 0x7FFF + ((b >> 16) & 1)
    return (rounded >> 16).astype(np.uint16)


def _build_shingle_u16(vol):
    """S[x, y, z, c, dx, dy] = vol[c, x+dx, y+dy, z], flat [NROW, 128] u16."""
    v = np.ascontiguousarray(np.asarray(vol[0], dtype=np.float32))  # (C,X,Y,Z)
    vb = _f32_to_bf16_bits(v)
    vt = np.ascontiguousarray(vb.transpose(1, 2, 3, 0))             # (X,Y,Z,C)
    S = np.zeros((X, Y, Z, C, 4, 4), np.uint16)
    for dx in range(4):
        for dy in range(4):
            S[:X - dx, :Y - dy, :, :, dx, dy] = vt[dx:, dy:, :, :]
    return S.reshape(X * Y * Z, 128)


def _host_rows(vert):
    """Exact replica of the device clip/floor -> shingle row per vertex."""
    v = np.asarray(vert[0], dtype=np.float32)
    vc = np.empty_like(v)
    for d, dim in enumerate((X, Y, Z)):
        vc[:, d] = np.clip(v[:, d], np.float32(1.0 + 1e-5),
                           np.float32(dim - 2 - 1e-5))
    mg = np.float32(MAGIC)
    fl = ((vc - np.float32(0.5)) + mg) - mg
    fli = fl.astype(np.int64)
    return ((fli[:, 0] - 1) * Y + (fli[:, 1] - 1)) * Z + (fli[:, 2] - 1)


def _prepare(vert, vol):
    rows = _host_rows(vert)                      # (V,)
    order = np.argsort(rows, kind="stable")
    Sfull = _build_shingle_u16(vol)

    # per-core grouping
    cores = []
    nslab_needed = 0
    for c in range(NCORES):
        ids = order[c * VCORE:(c + 1) * VCORE]
        r = rows[ids]
        groups = []
        i = 0
        n = len(ids)
        while i < n:
            jmax = min(i + GROUP, n)
            j = int(np.searchsorted(r, r[i] + SPAN_CAP, side="right"))
            j = min(j, jmax)
            groups.append((i, j))
            i = j
        cores.append((ids, r, groups))
        nslab_needed = max(nslab_needed, len(groups))
    nslab = nslab_needed
    m = nslab * GCOLS
    srows = nslab * BSTRIDE

    vert_f = np.asarray(vert[0], dtype=np.float32)
    in_maps = []
    perms = []   # per core: (orig_ids, flat positions in out)
    for c in range(NCORES):
        ids, r, groups = cores[c]
        sh = np.zeros((srows, P), np.uint16)
        vpad = np.full((m * P, 3), 1.5, np.float32)
        idx16 = np.zeros((nslab, 16, GROUP // 16), np.int16)
        flat_pos = np.empty(len(ids), np.int64)
        for b in range(nslab):
            if b < len(groups):
                i, j = groups[b]
                r0 = int(r[i])
                r1 = int(r[j - 1])
                nrow = min(r1 - r0 + 4, srows - b * BSTRIDE)
                sh[b * BSTRIDE:b * BSTRIDE + nrow] = Sfull[r0:r0 + nrow]
                rel = (r[i:j] - r0).astype(np.int16)
                cnt = j - i
            else:
                rel = np.zeros(0, np.int16)
                cnt = 0
            jj = np.arange(cnt)
            # index position within call: wrapped [16, GROUP//16]
            full_rel = np.zeros(GROUP, np.int16)
            full_rel[:cnt] = rel
            idx16[b] = full_rel.reshape(GROUP // 16, 16).T
            if cnt:
                gi = groups[b][0]
                # vertex ids[gi + jj] lands at partition jj%128, col b*8+jj//128
                part = jj % P
                col = b * GCOLS + jj // P
                flat_pos[gi:gi + cnt] = part * m + col
                vpad[part * m + col] = vert_f[ids[gi:gi + cnt]]
        in_maps.append({
            "shingle": sh.view(ml_dtypes.bfloat16),
            "vert": np.ascontiguousarray(vpad.reshape(P, m * 3)),
            "idx": np.ascontiguousarray(
                np.tile(idx16.transpose(1, 0, 2).reshape(16, -1), (8, 1))),
        })
        perms.append((ids, flat_pos))
    return nslab, in_maps, perms


def run_cores(vert, vol, trace=False, n_cores=NCORES, **kwargs):
    nslab, in_maps, perms = _prepare(vert, vol)
    nc = _get_program(nslab)
    res = run_bass_kernel_spmd(nc, in_maps, list(range(n_cores)),
                               trace=trace, **kwargs)
    m = nslab * GCOLS
    full = np.zeros((1, V, C), np.float32)
    for c in range(n_cores):
        out = np.asarray(res.results[c]["out"]).reshape(P * m, C)
        ids, flat_pos = perms[c]
        full[0, ids] = out[flat_pos]
    return full, res


def kernel(vert, vol):
    full, _ = run_cores(vert, vol, trace=False)
    return full
